# revision 15
# baseline (speedup 1.0000x reference)
"""Trainium2 Bass kernel for nn_CliffordRollingAttention.

Strategy (head-parallel over 8 cores, 2 heads/core):
  - Host pre-transposes x -> xT [D, B*L] bf16, slices/folds weights per core.
  - On-device per core:
      P1: QKV projections on PE in transposed layout [d, l] (stationary =
          weight tiles, moving = xT), bf16 with fp32 PSUM accumulation.
          Per-row sumsq partials for RMS norm via ACT Square + PE ones-reduce.
      P2: 64KB AllReduce of q/k sumsq partials across the 8 cores,
          rsqrt in column-form.
      P3: rms broadcast via K=1 matmul; normalize k; channel-roll score
          mixing folded into a host-built 128x128 matrix (Pm) -> one matmul.
      P4: scores: DVE products qm * k[:, l-s] (seq shift = free-dim offset)
          + PE one-hot-column reduce accumulating [16 shifts, 512 l] in PSUM.
          Max-free softmax (|logit| <= ~4.2): ACT exp, PE ones denom.
      P5: PE-transpose exp rows to row-major [l, 17], normalize attn.
      P6: apply-V row-major: v re-read shifted via DMA-transpose from a
          transposed DRAM copy; fused scalar_tensor_tensor accumulation
          in two bf16 chains of 8 shifts, merged in fp32.
      P7: output projection partial (this core's 256 channels), bf16 out.
  - Host sums the 8 partials in fp32 and adds the output bias.
"""

import numpy as np
import ml_dtypes

import concourse.bass as bass
import concourse.bacc as bacc
import concourse.mybir as mybir
import concourse.tile as tile
from concourse.bass_utils import run_bass_kernel_spmd

BF = ml_dtypes.bfloat16
FP32 = mybir.dt.float32
BF16 = mybir.dt.bfloat16

B, L, D = 2, 4096, 2048
H, DH = 16, 128
NCORES = 8
HPC = H // NCORES          # heads per core = 2
DPC = HPC * DH             # channels per core = 256
N = B * L                  # 8192 rows
EPS = 1e-6
SEQ_SHIFTS = [0, 1, -1, 3, -3, 9, -9, 26, -26, 78, -78, 232, -232, 689, -689, 2048]
CH_SHIFTS = [1, 2, 4, 8]
NS = len(SEQ_SHIFTS)       # 16
CHUNK = 512
NCHUNK = N // CHUNK        # 16
NLT = N // 128             # 64 l-tiles
AluOp = mybir.AluOpType
AF = mybir.ActivationFunctionType


def _wrap_runs(start, length):
    """Split output positions j in [0,length) whose source row is
    b*L + ((start_local + j) mod L), into maximal contiguous source runs.
    `start` is an absolute flattened row index that may be out of range
    within its batch. The batch is determined by the caller; here `start`
    is already batch-local (can be negative or >= L). Returns list of
    (j_offset, src_local_start, run_len)."""
    runs = []
    j = 0
    while j < length:
        src = (start + j) % L
        run = min(length - j, L - src)
        runs.append((j, src, run))
        j += run
    return runs


def _build_program():
    nc = bacc.Bacc(num_devices=NCORES)

    handles = {
        "xT": nc.declare_dram_parameter("xT", [D, N], BF16, isOutput=False),
        "wT": nc.declare_dram_parameter("wT", [D, 6 * 128], BF16, isOutput=False),
        "bias6": nc.declare_dram_parameter("bias6", [128, 6], FP32, isOutput=False),
        "pmT": nc.declare_dram_parameter("pmT", [128, HPC * 128], BF16, isOutput=False),
        "oh": nc.declare_dram_parameter("oh", [128, NS * 16], BF16, isOutput=False),
        "idm": nc.declare_dram_parameter("idm", [128, 128], BF16, isOutput=False),
        "ones_r": nc.declare_dram_parameter("ones_r", [1, 128], FP32, isOutput=False),
        "woT": nc.declare_dram_parameter("woT", [DPC, D], BF16, isOutput=False),
        "outp": nc.declare_dram_parameter("outp", [N, D], BF16, isOutput=True),
    }

    import contextlib
    with tile.TileContext(nc) as tc:
        with contextlib.ExitStack() as ctx:
            _emit_inner(ctx, tc, handles)
    nc.compile()
    return nc


def _emit_inner(ctx, tc, handles):
    nc = tc.nc
    xT = handles["xT"][:]
    wT = handles["wT"][:]
    bias6 = handles["bias6"][:]
    pmT_d = handles["pmT"][:]
    oh_d = handles["oh"][:]
    idm_d = handles["idm"][:]
    ones_r_d = handles["ones_r"][:]
    woT_d = handles["woT"][:]
    outp = handles["outp"][:]

    # ---------------- persistent pools ----------------
    const = ctx.enter_context(tc.tile_pool(name="const", bufs=1))
    big = ctx.enter_context(tc.tile_pool(name="big", bufs=1))
    dram = ctx.enter_context(tc.tile_pool(name="dram", bufs=1, space="DRAM"))

    w_sb = const.tile([128, 16 * 768], BF16)        # 24KB
    bias_sb = const.tile([128, 6], FP32)
    pm_sb = const.tile([128, HPC * 128], BF16)
    oh_sb = const.tile([128, NS * 16], BF16)
    id_sb = const.tile([128, 128], BF16)
    onesr_sb = const.tile([1, 128], FP32)
    eps_sb = const.tile([128, 1], FP32)
    wo_sb = const.tile([128, HPC * D], BF16)        # 8KB: [128, 2*2048] dtile-major

    # wT [2048, 768] -> w_sb[p, k*768+j] = wT[128k+p, j]
    nc.sync.dma_start(w_sb[:].rearrange("p (k j) -> p k j", k=16),
                      wT.rearrange("(k p) j -> p k j", p=128))
    nc.sync.dma_start(bias_sb[:], bias6)
    nc.sync.dma_start(pm_sb[:], pmT_d)
    nc.sync.dma_start(oh_sb[:], oh_d)
    nc.sync.dma_start(id_sb[:], idm_d)
    nc.sync.dma_start(onesr_sb[:], ones_r_d)
    nc.gpsimd.memset(eps_sb[:], EPS)
    # woT [256, 2048] -> [p, dt*2048 + e]
    nc.sync.dma_start(wo_sb[:].rearrange("p (dt e) -> p dt e", dt=2),
                      woT_d.rearrange("(dt p) e -> p dt e", p=128))

    q_raw = big.tile([128, HPC * N], BF16, tag="qraw")   # 32KB  [p, h*N + l]
    k_sb = big.tile([128, HPC * N], BF16)                # 32KB
    qm_sb = big.tile([128, HPC * N], BF16)               # 32KB
    exp_sb = big.tile([16, HPC * N], BF16)               # 32KB  [s, h*N + l]
    attn_all = big.tile([128, NLT * HPC * 16], FP32)     # 4KB   [p, (t*2+h)*16 + i]

    vT_dram = dram.tile([DPC, N], BF16)
    ss_dram = dram.tile([2, N], FP32)
    ss_out = dram.tile([2, N], FP32)
    rms_dram = dram.tile([2, N], FP32)

    # ---------------- P1: projections ----------------
    with tc.tile_pool(name="p1x", bufs=3) as p1x, \
         tc.tile_pool(name="p1ps", bufs=1, space="PSUM") as p1ps, \
         tc.tile_pool(name="p1ss", bufs=1, space="PSUM") as p1ss, \
         tc.tile_pool(name="p1sc", bufs=3) as p1sc:
        for c in range(NCHUNK):
            cs = c * CHUNK
            psums = [p1ps.tile([128, CHUNK], FP32, tag=f"proj{m}", name=f"proj{m}_{c}") for m in range(6)]
            for k in range(16):
                xt = p1x.tile([128, CHUNK], BF16, tag="xt")
                nc.sync.dma_start(xt[:], xT[128 * k:128 * (k + 1), cs:cs + CHUNK])
                for m in range(6):
                    nc.tensor.matmul(
                        psums[m][:],
                        w_sb[:, k * 768 + 128 * m: k * 768 + 128 * (m + 1)],
                        xt[:],
                        start=(k == 0), stop=(k == 15),
                    )
            # m order: q0 q1 k0 k1 v0 v1
            ssps = {}
            for dt in range(2):
                # q: raw evict + square
                nc.scalar.activation(
                    q_raw[:, dt * N + cs: dt * N + cs + CHUNK], psums[dt][:],
                    AF.Identity, bias=bias_sb[:, dt:dt + 1])
                sq = p1sc.tile([128, CHUNK], BF16, tag="sq")
                nc.scalar.activation(sq[:], psums[dt][:], AF.Square,
                                     bias=bias_sb[:, dt:dt + 1])
                if dt == 0:
                    ssps['q'] = p1ss.tile([1, CHUNK], FP32, tag="ssq", name=f"ssq_{c}")
                nc.tensor.matmul(ssps['q'][:], oh_sb[:, 0:1], sq[:],
                                 start=(dt == 0), stop=(dt == 1))
                # k
                nc.scalar.activation(
                    k_sb[:, dt * N + cs: dt * N + cs + CHUNK], psums[2 + dt][:],
                    AF.Identity, bias=bias_sb[:, 2 + dt:3 + dt])
                sqk = p1sc.tile([128, CHUNK], BF16, tag="sqk")
                nc.scalar.activation(sqk[:], psums[2 + dt][:], AF.Square,
                                     bias=bias_sb[:, 2 + dt:3 + dt])
                if dt == 0:
                    ssps['k'] = p1ss.tile([1, CHUNK], FP32, tag="ssk", name=f"ssk_{c}")
                nc.tensor.matmul(ssps['k'][:], oh_sb[:, 0:1], sqk[:],
                                 start=(dt == 0), stop=(dt == 1))
                # v
                vst = p1sc.tile([128, CHUNK], BF16, tag="vst")
                nc.scalar.activation(vst[:], psums[4 + dt][:], AF.Identity,
                                     bias=bias_sb[:, 4 + dt:5 + dt])
                nc.sync.dma_start(
                    vT_dram[128 * dt:128 * (dt + 1), cs:cs + CHUNK], vst[:])
            ssr_q = p1sc.tile([1, CHUNK], FP32, tag="ssrq", name=f"ssrq_{c}")
            ssr_k = p1sc.tile([1, CHUNK], FP32, tag="ssrk", name=f"ssrk_{c}")
            nc.scalar.activation(ssr_q[:], ssps['q'][:], AF.Copy)
            nc.scalar.activation(ssr_k[:], ssps['k'][:], AF.Copy)
            nc.sync.dma_start(ss_dram[0:1, cs:cs + CHUNK], ssr_q[:])
            nc.sync.dma_start(ss_dram[1:2, cs:cs + CHUNK], ssr_k[:])

    # ---------------- P2: AllReduce + rsqrt ----------------
    nc.gpsimd.collective_compute(
        "AllReduce", AluOp.add,
        replica_groups=[list(range(NCORES))],
        ins=[ss_dram.opt()], outs=[ss_out.opt()],
    )
    with tc.tile_pool(name="p2", bufs=1) as p2:
        col = p2.tile([128, 128], FP32)
        nc.sync.dma_start(col[:, 0:64],
                          ss_out[0, :].rearrange("(t p) -> p t", p=128))
        nc.sync.dma_start(col[:, 64:128],
                          ss_out[1, :].rearrange("(t p) -> p t", p=128))
        srt = p2.tile([128, 128], FP32)
        nc.scalar.activation(srt[:], col[:], AF.Sqrt, bias=eps_sb[:], scale=1.0 / D)
        rinv = p2.tile([128, 128], FP32)
        nc.vector.reciprocal(rinv[:], srt[:])
        nc.sync.dma_start(rms_dram[0, :].rearrange("(t p) -> p t", p=128),
                          rinv[:, 0:64])
        nc.sync.dma_start(rms_dram[1, :].rearrange("(t p) -> p t", p=128),
                          rinv[:, 64:128])

    # ---------------- P3: normalize k, build qm ----------------
    with tc.tile_pool(name="p3r", bufs=2) as p3r, \
         tc.tile_pool(name="p3ps", bufs=2, space="PSUM") as p3ps, \
         tc.tile_pool(name="p3b", bufs=2) as p3b:
        for c in range(NCHUNK):
            cs = c * CHUNK
            rq_t = p3r.tile([1, CHUNK], FP32, tag="rq")
            rk_t = p3r.tile([1, CHUNK], FP32, tag="rk")
            nc.sync.dma_start(rq_t[:], rms_dram[0:1, cs:cs + CHUNK])
            nc.sync.dma_start(rk_t[:], rms_dram[1:2, cs:cs + CHUNK])
            rb_ps = p3ps.tile([128, CHUNK], FP32, tag="rbps")
            nc.tensor.matmul(rb_ps[:], onesr_sb[:], rq_t[:], start=True, stop=True)
            rqb = p3b.tile([128, CHUNK], BF16, tag="rqb")
            nc.scalar.activation(rqb[:], rb_ps[:], AF.Copy)
            rb_ps2 = p3ps.tile([128, CHUNK], FP32, tag="rbps")
            nc.tensor.matmul(rb_ps2[:], onesr_sb[:], rk_t[:], start=True, stop=True)
            rkb = p3b.tile([128, CHUNK], BF16, tag="rkb")
            nc.scalar.activation(rkb[:], rb_ps2[:], AF.Copy)
            for dt in range(2):
                sl = slice(dt * N + cs, dt * N + cs + CHUNK)
                nc.vector.tensor_tensor(k_sb[:, sl], k_sb[:, sl], rkb[:],
                                        op=AluOp.mult)
            for h in range(HPC):
                qm_ps = p3ps.tile([128, CHUNK], FP32, tag="qmps")
                nc.tensor.matmul(qm_ps[:], pm_sb[:, 128 * h:128 * (h + 1)],
                                 q_raw[:, h * N + cs: h * N + cs + CHUNK],
                                 start=True, stop=True)
                nc.vector.tensor_tensor(qm_sb[:, h * N + cs: h * N + cs + CHUNK],
                                        qm_ps[:], rqb[:], op=AluOp.mult)

    # ---------------- P4: scores + exp + denom ----------------
    with tc.tile_pool(name="p4p", bufs=3) as p4p, \
         tc.tile_pool(name="p4ps", bufs=2, space="PSUM") as p4ps:
        for c in range(NCHUNK):
            cs = c * CHUNK
            b = cs // L
            w0 = cs - b * L
            for h in range(HPC):
                sc_ps = p4ps.tile([16, CHUNK], FP32, tag="scps")
                for i, s in enumerate(SEQ_SHIFTS):
                    prod = p4p.tile([128, CHUNK], BF16, tag="prod")
                    for (joff, src, rl) in _wrap_runs(w0 - s, CHUNK):
                        nc.vector.tensor_tensor(
                            prod[:, joff:joff + rl],
                            qm_sb[:, h * N + cs + joff: h * N + cs + joff + rl],
                            k_sb[:, h * N + b * L + src: h * N + b * L + src + rl],
                            op=AluOp.mult)
                    nc.tensor.matmul(sc_ps[:], oh_sb[:, 16 * i:16 * (i + 1)],
                                     prod[:], start=(i == 0), stop=(i == NS - 1))
                nc.scalar.activation(exp_sb[0:16, h * N + cs: h * N + cs + CHUNK],
                                     sc_ps[:], AF.Exp)

    # ---------------- P5: transpose attn to row-major + normalize ----------
    with tc.tile_pool(name="p5ps", bufs=3, space="PSUM") as p5ps, \
         tc.tile_pool(name="p5s", bufs=3) as p5s:
        for t in range(NLT):
            for h in range(HPC):
                tr_ps = p5ps.tile([128, 16], BF16, tag="trps")
                nc.tensor.transpose(
                    tr_ps[:],
                    exp_sb[0:16, h * N + 128 * t: h * N + 128 * (t + 1)],
                    id_sb[0:16, 0:16])
                attx = p5s.tile([128, 16], FP32, tag="attx")
                nc.scalar.activation(attx[:], tr_ps[:], AF.Copy)
                dsum = p5s.tile([128, 1], FP32, tag="dsum")
                nc.vector.tensor_reduce(dsum[:], attx[:], axis=mybir.AxisListType.X,
                                        op=AluOp.add)
                rc = p5s.tile([128, 1], FP32, tag="rc")
                nc.vector.reciprocal(rc[:], dsum[:])
                nc.vector.tensor_scalar(
                    attn_all[:, (t * HPC + h) * 16: (t * HPC + h) * 16 + 16],
                    attx[:], rc[:], None, op0=AluOp.mult)

    # ---------------- P6: apply V ----------------
    with tc.tile_pool(name="p6v", bufs=6) as p6v, \
         tc.tile_pool(name="p6a", bufs=2) as p6a, \
         tc.tile_pool(name="p6o", bufs=3) as p6o:
        for t in range(NLT):
            b = (128 * t) // L
            w0 = 128 * t - b * L
            acc_a = p6a.tile([128, DPC], BF16, tag="acca")
            acc_b = p6a.tile([128, DPC], BF16, tag="accb")
            for i, s in enumerate(SEQ_SHIFTS):
                vr = p6v.tile([128, DPC], BF16, tag="vr")
                for (joff, src, rl) in _wrap_runs(w0 - s, 128):
                    if rl % 128 == 0:
                        nc.sync.dma_start(
                            vr[joff:joff + rl, :],
                            vT_dram[:, b * L + src: b * L + src + rl],
                            transpose=True)
                    else:
                        nc.sync.dma_start(
                            vr[joff:joff + rl, :],
                            vT_dram[:, b * L + src: b * L + src + rl]
                            .rearrange("d l -> l d"))
                acc = acc_a if i < 8 else acc_b
                first = i == 0 or i == 8
                for h in range(HPC):
                    a_col = (t * HPC + h) * 16 + i
                    sl = slice(128 * h, 128 * (h + 1))
                    if first:
                        nc.vector.tensor_scalar(
                            acc[:, sl], vr[:, sl],
                            attn_all[:, a_col:a_col + 1], None, op0=AluOp.mult)
                    else:
                        nc.vector.scalar_tensor_tensor(
                            acc[:, sl], vr[:, sl],
                            attn_all[:, a_col:a_col + 1], acc[:, sl],
                            op0=AluOp.mult, op1=AluOp.add)
            oa = p6o.tile([128, DPC], BF16, tag="oa")
            nc.vector.tensor_tensor(oa[:], acc_a[:], acc_b[:], op=AluOp.add)
            # transpose row-major [128 l, 256 d] -> out_attn_T [256 d, l]
            for dt in range(2):
                nc.sync.dma_start(
                    qm_sb[:, dt * N + 128 * t: dt * N + 128 * (t + 1)],
                    oa[:, 128 * dt:128 * (dt + 1)],
                    transpose=True)

    # out_attn_T lives in qm_sb (reuse: qm no longer needed)
    # ---------------- P7: output projection ----------------
    with tc.tile_pool(name="p7ps", bufs=4, space="PSUM") as p7ps, \
         tc.tile_pool(name="p7s", bufs=4) as p7s:
        for t in range(NLT):
            for e in range(4):
                ops = p7ps.tile([128, 512], FP32, tag="ops")
                for dt in range(2):
                    nc.tensor.matmul(
                        ops[:],
                        qm_sb[:, dt * N + 128 * t: dt * N + 128 * (t + 1)],
                        wo_sb[:, dt * D + 512 * e: dt * D + 512 * (e + 1)],
                        start=(dt == 0), stop=(dt == 1))
                ost = p7s.tile([128, 512], BF16, tag="ost")
                nc.scalar.activation(ost[:], ops[:], AF.Copy)
                nc.sync.dma_start(
                    outp[128 * t:128 * (t + 1), 512 * e:512 * (e + 1)], ost[:])


_PROG = None


def _get_program():
    global _PROG
    if _PROG is None:
        _PROG = _build_program()
    return _PROG


def _host_prep(inputs):
    x = np.asarray(inputs['x'], np.float32)
    wq = np.asarray(inputs['wq'], np.float32)
    wk = np.asarray(inputs['wk'], np.float32)
    wv = np.asarray(inputs['wv'], np.float32)
    bq = np.asarray(inputs['bq'], np.float32)
    bk = np.asarray(inputs['bk'], np.float32)
    bv = np.asarray(inputs['bv'], np.float32)
    qnw = np.asarray(inputs['q_norm_w'], np.float32)
    knw = np.asarray(inputs['k_norm_w'], np.float32)
    mix = np.asarray(inputs['score_mix_w'], np.float32)[0]
    wo = np.asarray(inputs['wo'], np.float32)

    xT = np.ascontiguousarray(x.reshape(N, D).T).astype(BF)
    scale = DH ** -0.5

    # one-hot column matrices for the shift-reduce, col 0 of block 0 = ones
    oh = np.zeros((128, NS * 16), np.float32)
    for i in range(NS):
        oh[:, 16 * i + i] = 1.0
    oh = oh.astype(BF)
    idm = np.eye(128, dtype=np.float32).astype(BF)
    ones_r = np.ones((1, 128), np.float32)

    in_maps = []
    for c in range(NCORES):
        cs = c * DPC
        sl = slice(cs, cs + DPC)
        wTc = np.concatenate([wq[sl].T, wk[sl].T, wv[sl].T], axis=1)  # [2048, 768]
        bias = np.stack([bq[cs:cs + 128], bq[cs + 128:cs + 256],
                         bk[cs:cs + 128], bk[cs + 128:cs + 256],
                         bv[cs:cs + 128], bv[cs + 128:cs + 256]], axis=1)  # [128, 6]
        # Pm per head: qm[d] = scale*knw[d]*sum_n mix_n*qnw[g(s_n(d))]*qraw[s_n(d)]
        pmT = np.zeros((128, HPC * 128), np.float32)  # [d', h*128 + d]
        for h in range(HPC):
            gh = c * HPC + h
            Pm = np.zeros((DH, DH), np.float32)
            for n, ch in enumerate([0] + CH_SHIFTS):
                for dd in range(DH):
                    dp = (dd - ch) % DH
                    Pm[dd, dp] += mix[n] * qnw[gh * DH + dp]
            Pm *= scale * knw[gh * DH:(gh + 1) * DH][:, None]
            pmT[:, 128 * h:128 * (h + 1)] = Pm.T
        woTc = np.ascontiguousarray(wo[:, sl].T)  # [256, 2048]
        in_maps.append({
            "xT": xT,
            "wT": wTc.astype(BF),
            "bias6": np.ascontiguousarray(bias),
            "pmT": pmT.astype(BF),
            "oh": oh,
            "idm": idm,
            "ones_r": ones_r,
            "woT": woTc.astype(BF),
        })
    return in_maps


LAST_RESULT = None


def kernel(**inputs):
    global LAST_RESULT
    import os
    in_maps = _host_prep(inputs)
    nc = _get_program()
    trace = bool(os.environ.get("CRA_TRACE"))
    res = run_bass_kernel_spmd(nc, in_maps, list(range(NCORES)), trace=trace)
    LAST_RESULT = res
    acc = np.zeros((N, D), np.float32)
    for r in res.results:
        acc += np.asarray(r["outp"], np.float32)
    acc += np.asarray(inputs['bo'], np.float32)
    return acc.reshape(B, L, D)
